# revision 40
# baseline (speedup 1.0000x reference)
"""Multi-head attention (B=4, N=2048, C=768, H=12, D=64) on 8 TRN2 NeuronCores.

Sharding: core c handles batch c//2 and query rows (c%2)*1024 .. +1024, all
heads. Each core recomputes K/V for its full batch. The host ROTATES x[b].T
per core so this core's query rows are always columns 0:1024 (attention is
permutation-invariant over keys).

Single-phase schedule: softmax exp on ScalarE is the hard floor (192
activations x ~1.1us = ~214us), so ALL projection work (qkv, v, proj) is
interleaved under the exp window as PE filler instead of running as a
separate 94us prefix phase:
  - a short prefix computes qT/kT band 0 only, so the first exp lands ~7us in
  - per key-tile step: scores for the head pair (row-tiled: head A on PE rows
    0:64, head B on rows 64:128, concurrent via tile_position auto-derive),
    phase-offset exps (exp-A finishes early, hiding psA refill latency so
    ACT never gaps with single-buffered score PSUM), AV of the previous pair
    riding 2-3 key-tiles per step, plus one qkv filler chain
  - PSUM: psA(2 banks) + psB(2) + pu(2) + filler(2) = 8 banks exactly
The 1/sqrt(D) scale is folded into the q columns of wqkvT on the host.
AV packs [v_h | ones] per key tile so the matmul also produces the softmax
denominator in PSUM row 64; normalization multiplies by the gpsimd-broadcast
reciprocal during PSUM->SBUF evict.
"""

from contextlib import ExitStack

import ml_dtypes
import numpy as np

import concourse.bass as bass
import concourse.tile as tile
from concourse import bacc, mybir
from concourse import bass_utils

B, N, C, H, Dh = 4, 2048, 768, 12, 64
P = 128
NCORES = 8
ROWS = N // 2  # query rows per core
SCALE = Dh ** -0.5

BF16 = mybir.dt.bfloat16
F32 = mybir.dt.float32

CB = C // P       # 6 contraction bands
NT = N // P       # 16 key tiles
QC = ROWS // 512  # 2 query half-chunks (N=512 matmuls)
NPAIR = H // 2    # 6 head pairs == bands

_cached_nc = None
LAST_RESULT = None  # BassKernelResults of the most recent run (for test harness)


def _build_nc():
    nc = bacc.Bacc(
        "TRN2",
        target_bir_lowering=False,
        debug=False,
        enable_asserts=False,
        num_devices=NCORES,
    )
    import os as _os
    DBG = bool(_os.environ.get("KDBG"))
    xT_d = nc.dram_tensor("xT", [C, N], BF16, kind="ExternalInput")
    wqkvT_d = nc.dram_tensor("wqkvT", [C, 3 * C], BF16, kind="ExternalInput")
    wprojT_d = nc.dram_tensor("wprojT", [C, C], BF16, kind="ExternalInput")
    bproj_d = nc.dram_tensor("bproj", [CB, P, 1], F32, kind="ExternalInput")
    out_d = nc.dram_tensor("out", [C, ROWS], F32, kind="ExternalOutput")
    if DBG:
        dbg_qT = nc.dram_tensor("dbg_qT", [C, ROWS], BF16, kind="ExternalOutput")
        dbg_kT = nc.dram_tensor("dbg_kT", [C, N], BF16, kind="ExternalOutput")
        dbg_vo = nc.dram_tensor("dbg_vo", [N, H * (Dh + 1)], BF16, kind="ExternalOutput")
        dbg_att = nc.dram_tensor("dbg_att", [C, ROWS], BF16, kind="ExternalOutput")
        dbg_ut = nc.dram_tensor("dbg_ut", [2, P, ROWS], BF16, kind="ExternalOutput")
        dbg_pu = nc.dram_tensor("dbg_pu", [H, 65, ROWS], F32, kind="ExternalOutput")

    Exp = mybir.ActivationFunctionType.Exp

    with tile.TileContext(nc) as tc:
        with ExitStack() as ctx:
            # ---- persistent SBUF pools ----
            pool_w = ctx.enter_context(tc.tile_pool(name="wqkv", bufs=1))
            pool_x = ctx.enter_context(tc.tile_pool(name="xT", bufs=1))
            pool_wp = ctx.enter_context(tc.tile_pool(name="wproj", bufs=1))
            pool_bias = ctx.enter_context(tc.tile_pool(name="bias", bufs=1))
            pool_qT = ctx.enter_context(tc.tile_pool(name="qT", bufs=1))
            pool_kT = ctx.enter_context(tc.tile_pool(name="kT", bufs=1))
            pool_vo = ctx.enter_context(tc.tile_pool(name="vones", bufs=1))
            pool_attT = ctx.enter_context(tc.tile_pool(name="attT", bufs=1))
            pool_ua = ctx.enter_context(tc.tile_pool(name="ua", bufs=10))
            pool_ub = ctx.enter_context(tc.tile_pool(name="ub", bufs=18))
            pool_st = ctx.enter_context(tc.tile_pool(name="st", bufs=1))
            pool_r = ctx.enter_context(tc.tile_pool(name="r", bufs=2))
            pool_rb = ctx.enter_context(tc.tile_pool(name="rb", bufs=1))
            pool_y = ctx.enter_context(tc.tile_pool(name="y", bufs=2))
            # ---- PSUM pools: 2+2+2+2 = 8 banks ----
            ps_a = ctx.enter_context(tc.tile_pool(name="ps_a", bufs=1, space="PSUM"))
            ps_b = ctx.enter_context(tc.tile_pool(name="ps_b", bufs=1, space="PSUM"))
            ps_u = ctx.enter_context(tc.tile_pool(name="ps_u", bufs=1, space="PSUM"))
            ps_f = ctx.enter_context(tc.tile_pool(name="ps_f", bufs=2, space="PSUM"))

            wqkv_sb = [pool_w.tile([P, 3 * C], BF16, name=f"wqkv{i}") for i in range(CB)]
            x_sb = [pool_x.tile([P, N], BF16, name=f"x{i}") for i in range(CB)]
            wp_sb = [pool_wp.tile([P, C], BF16, name=f"wp{i}") for i in range(CB)]
            bias_sb = [pool_bias.tile([P, 1], F32, name=f"bias{i}") for i in range(CB)]
            qT_sb = [pool_qT.tile([P, ROWS], BF16, name=f"qT{i}") for i in range(CB)]
            kT_sb = [pool_kT.tile([P, N], BF16, name=f"kT{i}") for i in range(CB)]
            vo_sb = [pool_vo.tile([P, H * (Dh + 1)], BF16, name=f"vo{i}") for i in range(NT)]
            attT_sb = [pool_attT.tile([P, ROWS], BF16, name=f"attT{i}") for i in range(CB)]

            for nt in range(NT):
                nc.gpsimd.memset(vo_sb[nt][:], 1.0)

            # ---- DMA emission in need-order (pair order is 4,5,0,1,2,3).
            # Descriptor ISSUE costs ~620ns each on the Sync queue, so the
            # six x query-col loads (the first-scores gate) go first.
            B4 = 4 * P
            for cb in range(CB):
                nc.sync.dma_start(x_sb[cb][:, 0:ROWS], xT_d.ap()[cb * P:(cb + 1) * P, 0:ROWS])
            for cb in range(CB):
                nc.sync.dma_start(wqkv_sb[cb][:, B4:B4 + P], wqkvT_d.ap()[cb * P:(cb + 1) * P, B4:B4 + P])
                nc.sync.dma_start(wqkv_sb[cb][:, C + B4:C + B4 + P], wqkvT_d.ap()[cb * P:(cb + 1) * P, C + B4:C + B4 + P])
            # 3) v chunk-1 weights (heads 8-11: earliest v fillers)
            for cb in range(CB):
                nc.sync.dma_start(wqkv_sb[cb][:, 2 * C + 512:3 * C], wqkvT_d.ap()[cb * P:(cb + 1) * P, 2 * C + 512:3 * C])
            # 4) x key-cols (keys 1024:2047)
            for cb in range(CB):
                nc.sync.dma_start(x_sb[cb][:, ROWS:N], xT_d.ap()[cb * P:(cb + 1) * P, ROWS:N])
            # 5) band-5 q/k weights, v chunk-0, remaining q/k bands
            B5 = 5 * P
            for cb in range(CB):
                nc.sync.dma_start(wqkv_sb[cb][:, B5:B5 + P], wqkvT_d.ap()[cb * P:(cb + 1) * P, B5:B5 + P])
                nc.sync.dma_start(wqkv_sb[cb][:, C + B5:C + B5 + P], wqkvT_d.ap()[cb * P:(cb + 1) * P, C + B5:C + B5 + P])
            for cb in range(CB):
                nc.sync.dma_start(wqkv_sb[cb][:, 2 * C:2 * C + 512], wqkvT_d.ap()[cb * P:(cb + 1) * P, 2 * C:2 * C + 512])
            for cb in range(CB):
                nc.sync.dma_start(wqkv_sb[cb][:, 0:B4], wqkvT_d.ap()[cb * P:(cb + 1) * P, 0:B4])
                nc.sync.dma_start(wqkv_sb[cb][:, C:C + B4], wqkvT_d.ap()[cb * P:(cb + 1) * P, C:C + B4])
            # 6) proj weights + bias (needed last)
            for cb in range(CB):
                nc.sync.dma_start(wp_sb[cb][:], wprojT_d.ap()[cb * P:(cb + 1) * P, :])
                nc.sync.dma_start(bias_sb[cb][:], bproj_d.ap()[cb, :, :])

            # ---- PE warmup: HAM needs ~3.4us of activity to unthrottle.
            # Dummy matmuls on the first-arriving weight tile keep the PE busy
            # while x streams in, so prefix chains run at 2.4 GHz.
            wu = ps_f.tile([P, 512], F32, name="warm", tag="fill")
            for i in range(24):
                nc.tensor.matmul(
                    wu[:, 0:P], wqkv_sb[0][:, B4:B4 + P], wqkv_sb[0][:, B4:B4 + P],
                    start=True, stop=True,
                )

            # ---- chain emitters. Each qkv chain is split into two filler
            # "parts" (3 contraction bands each, ~0.64us) so per-step filler
            # granularity matches the ~0.6us/step PE slack.
            def gemm_parts(stat_fn, mov_fn, evict_fn, name, width=512):
                state = {}

                def part(lo, hi, first, last):
                    if first:
                        state["pt"] = ps_f.tile([P, 512], F32, name=name, tag="fill")
                    pt = state["pt"]
                    for cb in range(lo, hi):
                        nc.tensor.matmul(
                            pt[:, 0:width],
                            stat_fn(cb),
                            mov_fn(cb),
                            start=(cb == 0),
                            stop=(cb == CB - 1),
                        )
                    if last:
                        evict_fn(pt)

                return [lambda: part(0, 3, True, False), lambda: part(3, CB, False, True)]

            def qT_parts(ob, qc):
                return gemm_parts(
                    lambda cb: wqkv_sb[cb][:, ob * P:(ob + 1) * P],
                    lambda cb: x_sb[cb][:, qc * 512:(qc + 1) * 512],
                    lambda pt: nc.vector.tensor_copy(
                        qT_sb[ob][:, qc * 512:(qc + 1) * 512], pt[:]),
                    f"qT{ob}{qc}",
                )

            def kT_parts(ob, col0, width=512):
                return gemm_parts(
                    lambda cb: wqkv_sb[cb][:, C + ob * P:C + (ob + 1) * P],
                    lambda cb: x_sb[cb][:, col0:col0 + width],
                    lambda pt: nc.vector.tensor_copy(
                        kT_sb[ob][:, col0:col0 + width], pt[:, 0:width]),
                    f"kT{ob}_{col0}",
                    width=width,
                )

            def v_parts(nt, chunk):
                off, width = ((0, 512), (512, 256))[chunk]
                h0, hn = (0, 8) if chunk == 0 else (8, 4)
                return gemm_parts(
                    lambda cb: x_sb[cb][:, nt * P:(nt + 1) * P],
                    lambda cb: wqkv_sb[cb][:, 2 * C + off:2 * C + off + width],
                    lambda pt: nc.vector.tensor_copy(
                        vo_sb[nt].rearrange("p (h e) -> p h e", e=Dh + 1)[:, h0:h0 + hn, 0:Dh],
                        pt[:, 0:width].rearrange("p (h e) -> p h e", e=Dh)),
                    f"v{nt}{chunk}",
                    width=width,
                )

            def emit_all(parts):
                for fn in parts:
                    fn()

            # ---- prefix: band-4 qT + kt0-covering kT slice; scores can
            # start as soon as the query columns of x land.
            emit_all(qT_parts(4, 0))
            emit_all(qT_parts(4, 1))
            emit_all(kT_parts(4, 0, 256))

            # ---- filler queue, deadline-ordered for pair order 4,5,0,1,2,3.
            # band_end[b] = filler index that must be emitted before band-b
            # scores (reads must not precede writes in emission order).
            v_emit = [[-1] * NT, [-1] * NT]  # [chunk][nt] -> filler index
            fillers = []  # (emit_fn, (chunk, nt) completed or None)
            band_end = [0] * CB

            def add(parts, done=None):
                for fn in parts[:-1]:
                    fillers.append((fn, None))
                fillers.append((parts[-1], done))

            add(kT_parts(4, 256, 256))
            add(kT_parts(4, 512))
            for nt in range(4):
                add(v_parts(nt, 1), (1, nt))
            add(kT_parts(4, 1024))
            add(kT_parts(4, 1536))
            for nt in range(4, NT):
                add(v_parts(nt, 1), (1, nt))
            for b in (5, 0):
                for kc in range(2):
                    for half in range(2):
                        add(kT_parts(b, kc * 1024 + half * 512))
                add(qT_parts(b, 0))
                add(qT_parts(b, 1))
                band_end[b] = len(fillers)
            for nt in range(NT):
                add(v_parts(nt, 0), (0, nt))
            for b in (1, 2, 3):
                for kc in range(2):
                    for half in range(2):
                        add(kT_parts(b, kc * 1024 + half * 512))
                add(qT_parts(b, 0))
                add(qT_parts(b, 1))
                band_end[b] = len(fillers)
            fill_i = [0]

            def pop_fill(k=1):
                for _ in range(k):
                    if fill_i[0] < len(fillers):
                        fn, done = fillers[fill_i[0]]
                        fn()
                        if done is not None:
                            v_emit[done[0]][done[1]] = fill_i[0]
                        fill_i[0] += 1

            def pop_fill_until(idx):
                while fill_i[0] < idx:
                    pop_fill(1)

            # ---- attention state ----
            uts = {}   # (h, kt) -> ut tile
            pus = {}   # h -> pu accumulator

            def emit_scores(p, kt):
                band = p
                psA = ps_a.tile([P, 1024], F32, name="psA", tag="ps")
                for qc in range(QC):
                    nc.tensor.matmul(
                        psA[:, qc * 512:(qc + 1) * 512],
                        kT_sb[band][0:64, kt * P:(kt + 1) * P],
                        qT_sb[band][0:64, qc * 512:(qc + 1) * 512],
                        start=True,
                        stop=True,
                    )
                psB = ps_b.tile([P, 1024], F32, name="psB", tag="ps")
                for qc in range(QC):
                    nc.tensor.matmul(
                        psB[:, qc * 512:(qc + 1) * 512],
                        kT_sb[band][64:128, kt * P:(kt + 1) * P],
                        qT_sb[band][64:128, qc * 512:(qc + 1) * 512],
                        start=True,
                        stop=True,
                    )
                utA = pool_ua.tile([P, 1024], BF16, name="utA", tag="ua")
                nc.scalar.activation(utA[:], psA[:], Exp)
                utB = pool_ub.tile([P, 1024], BF16, name="utB", tag="ub")
                nc.scalar.activation(utB[:], psB[:], Exp)
                uts[(2 * p, kt)] = utA
                uts[(2 * p + 1, kt)] = utB
                if DBG and p == 4 and kt == 0:
                    nc.sync.dma_start(dbg_ut.ap()[0], utA[:])
                    nc.sync.dma_start(dbg_ut.ap()[1], utB[:])

            def emit_av(h, kt):
                if kt == 0:
                    pus[h] = ps_u.tile([65, 1024], F32, name="pu")
                pu = pus[h]
                for qc in range(QC):
                    nc.tensor.matmul(
                        pu[0:65, qc * 512:(qc + 1) * 512],
                        vo_sb[kt][:, h * 65:(h + 1) * 65],
                        uts[(h, kt)][:, qc * 512:(qc + 1) * 512],
                        start=(kt == 0),
                        stop=(kt == NT - 1),
                    )
                del uts[(h, kt)]

            def emit_normalize(h):
                # copy pu -> SBUF staging first: frees the single pu PSUM
                # buffer after ~1.2us instead of after the full chain
                band, hp = divmod(h, 2)
                po = hp * 64
                pu = pus.pop(h)
                st = pool_st.tile([64, ROWS], BF16, name="st")
                nc.vector.tensor_copy(st[:], pu[0:64, :])
                s = pool_r.tile([1, ROWS], F32, name="s", tag="r")
                nc.vector.tensor_copy(s[:], pu[64:65, :])
                r = pool_r.tile([1, ROWS], F32, name="r", tag="r")
                nc.vector.reciprocal_approx_fast(r[:], s[:])
                rb = pool_rb.tile([64, ROWS], F32, name="rb")
                nc.gpsimd.partition_broadcast(rb[:], r[:])
                nc.vector.tensor_mul(attT_sb[band][po:po + 64, :], st[:], rb[:])

            # ---- main loop. Per step: AV rides FIRST (deadlock-safe: a
            # stalled ride can never block the exp that frees its ut slot),
            # then filler, then scores (phase-offset exps keep ACT gap-free).
            # pu (single PSUM accumulator) rotation per pair p:
            #   step 0:    finish head A(p-1) spill kts, copy pu -> staging
            #   steps 1-8: ride head B(p-1), 2 kts/step; then copy/normalize
            #   steps 9-15: self-ride head A(p) (its exps are >=1 step old)
            av_done = [0] * H     # kts consumed per head
            norm_done = [False] * H

            def finish(h):
                if not norm_done[h]:
                    emit_normalize(h)
                    norm_done[h] = True

            def ride(h, want, kt_cap=NT):
                # emit up to `want` AV kts for head h; the vo chunk this head
                # lives in must have been emitted; self-rides capped at kt<step
                chunk = 0 if h < 8 else 1
                drained = fill_i[0] >= len(fillers)
                n = 0
                while n < want and av_done[h] < min(NT, kt_cap):
                    kt = av_done[h]
                    if v_emit[chunk][kt] < 0:
                        break
                    if not drained and fill_i[0] - v_emit[chunk][kt] < 2:
                        break
                    emit_av(h, kt)
                    av_done[h] += 1
                    n += 1
                if av_done[h] == NT:
                    finish(h)

            def force(h):
                # unconditional completion (the band deadlines guarantee the
                # vo chains for h were already emitted)
                while av_done[h] < NT:
                    emit_av(h, av_done[h])
                    av_done[h] += 1
                finish(h)

            PAIR_ORDER = [4, 5, 0, 1, 2, 3]
            cumB = [0, 3, 6, 9, 12, 14, 16]  # hB(prev) ride targets, steps 1-6
            for pi, p in enumerate(PAIR_ORDER):
                hA, hB = 2 * p, 2 * p + 1
                prev = PAIR_ORDER[pi - 1] if pi >= 1 else None
                prev2 = PAIR_ORDER[pi - 2] if pi >= 2 else None
                hAp = 2 * prev if prev is not None else None
                hBp = 2 * prev + 1 if prev is not None else None
                pop_fill_until(band_end[p])  # band deadline (usually no-op)
                for kt in range(NT):
                    # scores first: keeps the phase-offset exps gap-free
                    emit_scores(p, kt)
                    rode = 0
                    if pi > 0:
                        if kt == 0:
                            if prev2 is not None:
                                force(2 * prev2 + 1)  # safety: pu order
                            force(hAp)  # head A(prev) spill (kt 15)
                            rode = 2
                        elif kt <= 6:
                            n0 = av_done[hBp]
                            ride(hBp, max(0, cumB[kt] - n0))
                            rode += av_done[hBp] - n0
                        elif av_done[hBp] < NT:
                            n0 = av_done[hBp]
                            ride(hBp, NT)  # catch-up if vo gating lagged
                            rode += av_done[hBp] - n0
                    if kt >= 7 and (norm_done[hBp] if pi > 0 else True):
                        n0 = av_done[hA]
                        want = 2 * (kt - 6) - n0
                        ride(hA, max(0, want), kt_cap=kt + 1)
                        rode += av_done[hA] - n0
                    # adaptive filler pops: ~2.2us/step PE budget
                    if pi == 0 and kt <= 6:
                        pop_fill(2)
                    elif pi > 0 and kt <= 1:
                        pass  # boundary steps already carry force/ride bursts
                    else:
                        pop_fill(1 if rode >= 2 else 2)

            # ---- epilogue: finish last pair, overlap proj partials ----
            pop_fill(len(fillers))  # drain any leftovers
            force(2 * PAIR_ORDER[-2] + 1)
            pl = PAIR_ORDER[-1]
            hA, hB = 2 * pl, 2 * pl + 1
            while av_done[hA] < NT:
                emit_av(hA, av_done[hA])
                av_done[hA] += 1
            finish(hA)

            if DBG:
                for cb in range(CB):
                    nc.sync.dma_start(dbg_qT.ap()[cb * P:(cb + 1) * P, :], qT_sb[cb][:])
                    nc.sync.dma_start(dbg_kT.ap()[cb * P:(cb + 1) * P, :], kT_sb[cb][:])
                    nc.sync.dma_start(dbg_att.ap()[cb * P:(cb + 1) * P, :], attT_sb[cb][:])
                for nt in range(NT):
                    nc.sync.dma_start(dbg_vo.ap()[nt * P:(nt + 1) * P, :], vo_sb[nt][:])

            # ---- output projection: 12 chains in 2 groups of 6 (ps_f x2 +
            # ps_a + ps_b give 6 concurrent accumulators once scores stop).
            # Contraction order puts band 3 (normalized last) at the end of
            # each chain; group 0's partials interleave with h7's AV.
            PROJ_CB = (4, 5, 0, 1, 2, 3)
            chains = [(ob, qc) for ob in range(CB) for qc in range(QC)]

            def proj_mm(slot, ob, qc, i):
                cb = PROJ_CB[i]
                nc.tensor.matmul(
                    slot,
                    wp_sb[cb][:, ob * P:(ob + 1) * P],
                    attT_sb[cb][:, qc * 512:(qc + 1) * 512],
                    start=(i == 0),
                    stop=(i == CB - 1),
                )

            def proj_group(g, ride_h=None):
                base = 6 * g
                tf0 = ps_f.tile([P, 512], F32, name=f"pj{base}", tag="fill")
                tf1 = ps_f.tile([P, 512], F32, name=f"pj{base + 1}", tag="fill")
                ta = ps_a.tile([P, 1024], F32, name="pja", tag="ps")
                tb = ps_b.tile([P, 1024], F32, name="pjb", tag="ps")
                slots = [tf0[:], tf1[:], ta[:, 0:512], ta[:, 512:1024],
                         tb[:, 0:512], tb[:, 512:1024]]
                if ride_h is not None:
                    while av_done[ride_h] < NT:
                        emit_av(ride_h, av_done[ride_h])
                        av_done[ride_h] += 1
                    finish(ride_h)  # norm chain overlaps the part-1 matmuls
                for j in range(6):
                    ob, qc = chains[base + j]
                    for i in range(5):
                        proj_mm(slots[j], ob, qc, i)
                for j in range(6):
                    ob, qc = chains[base + j]
                    proj_mm(slots[j], ob, qc, 5)
                    y = pool_y.tile([P, 512], F32, name="y")
                    nc.vector.tensor_scalar_add(y[:], slots[j], bias_sb[ob][:])
                    nc.sync.dma_start(
                        out_d.ap()[ob * P:(ob + 1) * P, qc * 512:(qc + 1) * 512], y[:]
                    )

            proj_group(0, ride_h=hB)
            proj_group(1)

    nc.compile()
    return nc


def kernel(x, w_qkv, w_proj, b_proj):
    global _cached_nc, LAST_RESULT
    if _cached_nc is None:
        _cached_nc = _build_nc()
    nc = _cached_nc

    x = np.asarray(x, dtype=np.float32)
    w_qkv = np.asarray(w_qkv, dtype=np.float32)
    w_proj = np.asarray(w_proj, dtype=np.float32)
    b_proj = np.asarray(b_proj, dtype=np.float32)

    bf = ml_dtypes.bfloat16
    wqkvT = w_qkv.T.astype(np.float32).copy()  # [C, 3C]
    wqkvT[:, :C] *= SCALE  # fold q scaling
    wqkvT = np.ascontiguousarray(wqkvT).astype(bf)
    wprojT = np.ascontiguousarray(w_proj.T).astype(bf)
    bproj_dev = np.ascontiguousarray(b_proj.astype(np.float32).reshape(CB, P, 1))

    in_maps = []
    for c in range(NCORES):
        b, half = divmod(c, 2)
        xTb = x[b].T.astype(bf)  # [C, N]
        if half:
            xTb = np.roll(xTb, -ROWS, axis=1)  # query rows -> columns 0:1024
        in_maps.append(
            {
                "xT": np.ascontiguousarray(xTb),
                "wqkvT": wqkvT,
                "wprojT": wprojT,
                "bproj": bproj_dev,
            }
        )

    res = bass_utils.run_bass_kernel_spmd(nc, in_maps, core_ids=list(range(NCORES)))
    LAST_RESULT = res

    out = np.empty((B, N, C), np.float32)
    for c in range(NCORES):
        b, half = divmod(c, 2)
        out[b, half * ROWS:(half + 1) * ROWS, :] = res.results[c]["out"].T
    return out


# revision 41
# speedup vs baseline: 1.1844x; 1.1844x over previous
"""Multi-head attention (B=4, N=2048, C=768, H=12, D=64) on 8 TRN2 NeuronCores.

Sharding: core c handles batch c//2 and query rows (c%2)*1024 .. +1024, all
heads. Each core recomputes K/V for its full batch. The host ROTATES x[b].T
per core so this core's query rows are always columns 0:1024 (attention is
permutation-invariant over keys).

Single-phase schedule: softmax exp on ScalarE is the hard floor (192
activations x ~1.1us = ~214us), so ALL projection work (qkv, v, proj) is
interleaved under the exp window as PE filler instead of running as a
separate 94us prefix phase:
  - a short prefix computes qT/kT band 0 only, so the first exp lands ~7us in
  - per key-tile step: scores for the head pair (row-tiled: head A on PE rows
    0:64, head B on rows 64:128, concurrent via tile_position auto-derive),
    phase-offset exps (exp-A finishes early, hiding psA refill latency so
    ACT never gaps with single-buffered score PSUM), AV of the previous pair
    riding 2-3 key-tiles per step, plus one qkv filler chain
  - PSUM: psA(2 banks) + psB(2) + pu(2) + filler(2) = 8 banks exactly
The 1/sqrt(D) scale is folded into the q columns of wqkvT on the host.
AV packs [v_h | ones] per key tile so the matmul also produces the softmax
denominator in PSUM row 64; normalization multiplies by the gpsimd-broadcast
reciprocal during PSUM->SBUF evict.
"""

from contextlib import ExitStack

import ml_dtypes
import numpy as np

import concourse.bass as bass
import concourse.tile as tile
from concourse import bacc, mybir
from concourse import bass_utils

B, N, C, H, Dh = 4, 2048, 768, 12, 64
P = 128
NCORES = 8
ROWS = N // 2  # query rows per core
SCALE = Dh ** -0.5

BF16 = mybir.dt.bfloat16
F32 = mybir.dt.float32

CB = C // P       # 6 contraction bands
NT = N // P       # 16 key tiles
QC = ROWS // 512  # 2 query half-chunks (N=512 matmuls)
NPAIR = H // 2    # 6 head pairs == bands

_cached_nc = None
LAST_RESULT = None  # BassKernelResults of the most recent run (for test harness)


def _build_nc():
    nc = bacc.Bacc(
        "TRN2",
        target_bir_lowering=False,
        debug=False,
        enable_asserts=False,
        num_devices=NCORES,
    )
    import os as _os
    DBG = bool(_os.environ.get("KDBG"))
    xT_d = nc.dram_tensor("xT", [C, N], BF16, kind="ExternalInput")
    wqkvT_d = nc.dram_tensor("wqkvT", [C, 3 * C], BF16, kind="ExternalInput")
    wprojT_d = nc.dram_tensor("wprojT", [C, C], BF16, kind="ExternalInput")
    bproj_d = nc.dram_tensor("bproj", [CB, P, 1], F32, kind="ExternalInput")
    out_d = nc.dram_tensor("out", [C, ROWS], F32, kind="ExternalOutput")
    if DBG:
        dbg_qT = nc.dram_tensor("dbg_qT", [C, ROWS], BF16, kind="ExternalOutput")
        dbg_kT = nc.dram_tensor("dbg_kT", [C, N], BF16, kind="ExternalOutput")
        dbg_vo = nc.dram_tensor("dbg_vo", [N, H * (Dh + 1)], BF16, kind="ExternalOutput")
        dbg_att = nc.dram_tensor("dbg_att", [C, ROWS], BF16, kind="ExternalOutput")
        dbg_ut = nc.dram_tensor("dbg_ut", [2, P, ROWS], BF16, kind="ExternalOutput")
        dbg_pu = nc.dram_tensor("dbg_pu", [H, 65, ROWS], F32, kind="ExternalOutput")

    Exp = mybir.ActivationFunctionType.Exp

    with tile.TileContext(nc) as tc:
        with ExitStack() as ctx:
            # ---- persistent SBUF pools ----
            pool_w = ctx.enter_context(tc.tile_pool(name="wqkv", bufs=1))
            pool_x = ctx.enter_context(tc.tile_pool(name="xT", bufs=1))
            pool_wp = ctx.enter_context(tc.tile_pool(name="wproj", bufs=1))
            pool_bias = ctx.enter_context(tc.tile_pool(name="bias", bufs=1))
            pool_qT = ctx.enter_context(tc.tile_pool(name="qT", bufs=1))
            pool_kT = ctx.enter_context(tc.tile_pool(name="kT", bufs=1))
            pool_vo = ctx.enter_context(tc.tile_pool(name="vones", bufs=1))
            pool_attT = ctx.enter_context(tc.tile_pool(name="attT", bufs=1))
            pool_ua = ctx.enter_context(tc.tile_pool(name="ua", bufs=10))
            pool_ub = ctx.enter_context(tc.tile_pool(name="ub", bufs=18))
            pool_st = ctx.enter_context(tc.tile_pool(name="st", bufs=1))
            pool_r = ctx.enter_context(tc.tile_pool(name="r", bufs=2))
            pool_rb = ctx.enter_context(tc.tile_pool(name="rb", bufs=1))
            pool_y = ctx.enter_context(tc.tile_pool(name="y", bufs=2))
            # ---- PSUM pools: 2+2+2+2 = 8 banks ----
            ps_a = ctx.enter_context(tc.tile_pool(name="ps_a", bufs=1, space="PSUM"))
            ps_b = ctx.enter_context(tc.tile_pool(name="ps_b", bufs=1, space="PSUM"))
            ps_u = ctx.enter_context(tc.tile_pool(name="ps_u", bufs=1, space="PSUM"))
            ps_f = ctx.enter_context(tc.tile_pool(name="ps_f", bufs=2, space="PSUM"))

            wqkv_sb = [pool_w.tile([P, 3 * C], BF16, name=f"wqkv{i}") for i in range(CB)]
            x_sb = [pool_x.tile([P, N], BF16, name=f"x{i}") for i in range(CB)]
            wp_sb = [pool_wp.tile([P, C], BF16, name=f"wp{i}") for i in range(CB)]
            bias_sb = [pool_bias.tile([P, 1], F32, name=f"bias{i}") for i in range(CB)]
            qT_sb = [pool_qT.tile([P, ROWS], BF16, name=f"qT{i}") for i in range(CB)]
            kT_sb = [pool_kT.tile([P, N], BF16, name=f"kT{i}") for i in range(CB)]
            vo_sb = [pool_vo.tile([P, H * (Dh + 1)], BF16, name=f"vo{i}") for i in range(NT)]
            attT_sb = [pool_attT.tile([P, ROWS], BF16, name=f"attT{i}") for i in range(CB)]

            for nt in range(NT):
                nc.gpsimd.memset(vo_sb[nt][:], 1.0)

            # ---- DMA emission in need-order (pair order is 4,5,0,1,2,3) ----
            # 1) band-4 q/k weights first (small; unblocks warmup + prefix)
            B4 = 4 * P
            for cb in range(CB):
                nc.sync.dma_start(wqkv_sb[cb][:, B4:B4 + P], wqkvT_d.ap()[cb * P:(cb + 1) * P, B4:B4 + P])
                nc.sync.dma_start(wqkv_sb[cb][:, C + B4:C + B4 + P], wqkvT_d.ap()[cb * P:(cb + 1) * P, C + B4:C + B4 + P])
            # 2) x query-cols (also keys 0:1023): prefix qT/kT chains
            for cb in range(CB):
                nc.sync.dma_start(x_sb[cb][:, 0:512], xT_d.ap()[cb * P:(cb + 1) * P, 0:512])
                nc.sync.dma_start(x_sb[cb][:, 512:ROWS], xT_d.ap()[cb * P:(cb + 1) * P, 512:ROWS])
            # 3) v chunk-1 weights (heads 8-11: earliest v fillers)
            for cb in range(CB):
                nc.sync.dma_start(wqkv_sb[cb][:, 2 * C + 512:3 * C], wqkvT_d.ap()[cb * P:(cb + 1) * P, 2 * C + 512:3 * C])
            # 4) x key-cols (keys 1024:2047)
            for cb in range(CB):
                nc.sync.dma_start(x_sb[cb][:, ROWS:N], xT_d.ap()[cb * P:(cb + 1) * P, ROWS:N])
            # 5) band-5 q/k weights, v chunk-0, remaining q/k bands
            B5 = 5 * P
            for cb in range(CB):
                nc.sync.dma_start(wqkv_sb[cb][:, B5:B5 + P], wqkvT_d.ap()[cb * P:(cb + 1) * P, B5:B5 + P])
                nc.sync.dma_start(wqkv_sb[cb][:, C + B5:C + B5 + P], wqkvT_d.ap()[cb * P:(cb + 1) * P, C + B5:C + B5 + P])
            for cb in range(CB):
                nc.sync.dma_start(wqkv_sb[cb][:, 2 * C:2 * C + 512], wqkvT_d.ap()[cb * P:(cb + 1) * P, 2 * C:2 * C + 512])
            for cb in range(CB):
                nc.sync.dma_start(wqkv_sb[cb][:, 0:B4], wqkvT_d.ap()[cb * P:(cb + 1) * P, 0:B4])
                nc.sync.dma_start(wqkv_sb[cb][:, C:C + B4], wqkvT_d.ap()[cb * P:(cb + 1) * P, C:C + B4])
            # 6) proj weights + bias (needed last)
            for cb in range(CB):
                nc.sync.dma_start(wp_sb[cb][:], wprojT_d.ap()[cb * P:(cb + 1) * P, :])
                nc.sync.dma_start(bias_sb[cb][:], bproj_d.ap()[cb, :, :])

            # ---- PE warmup: HAM needs ~3.4us of activity to unthrottle.
            # Dummy matmuls on the first-arriving weight tile keep the PE busy
            # while x streams in, so prefix chains run at 2.4 GHz.
            wu = ps_f.tile([P, 512], F32, name="warm", tag="fill")
            for i in range(24):
                nc.tensor.matmul(
                    wu[:, 0:P], wqkv_sb[0][:, B4:B4 + P], wqkv_sb[0][:, B4:B4 + P],
                    start=True, stop=True,
                )

            # ---- chain emitters. Each qkv chain is split into two filler
            # "parts" (3 contraction bands each, ~0.64us) so per-step filler
            # granularity matches the ~0.6us/step PE slack.
            def gemm_parts(stat_fn, mov_fn, evict_fn, name, width=512):
                state = {}

                def part(lo, hi, first, last):
                    if first:
                        state["pt"] = ps_f.tile([P, 512], F32, name=name, tag="fill")
                    pt = state["pt"]
                    for cb in range(lo, hi):
                        nc.tensor.matmul(
                            pt[:, 0:width],
                            stat_fn(cb),
                            mov_fn(cb),
                            start=(cb == 0),
                            stop=(cb == CB - 1),
                        )
                    if last:
                        evict_fn(pt)

                return [lambda: part(0, 3, True, False), lambda: part(3, CB, False, True)]

            def qT_parts(ob, qc):
                return gemm_parts(
                    lambda cb: wqkv_sb[cb][:, ob * P:(ob + 1) * P],
                    lambda cb: x_sb[cb][:, qc * 512:(qc + 1) * 512],
                    lambda pt: nc.vector.tensor_copy(
                        qT_sb[ob][:, qc * 512:(qc + 1) * 512], pt[:]),
                    f"qT{ob}{qc}",
                )

            def kT_parts(ob, col0, width=512):
                return gemm_parts(
                    lambda cb: wqkv_sb[cb][:, C + ob * P:C + (ob + 1) * P],
                    lambda cb: x_sb[cb][:, col0:col0 + width],
                    lambda pt: nc.vector.tensor_copy(
                        kT_sb[ob][:, col0:col0 + width], pt[:, 0:width]),
                    f"kT{ob}_{col0}",
                    width=width,
                )

            def v_parts(nt, chunk):
                off, width = ((0, 512), (512, 256))[chunk]
                h0, hn = (0, 8) if chunk == 0 else (8, 4)
                return gemm_parts(
                    lambda cb: x_sb[cb][:, nt * P:(nt + 1) * P],
                    lambda cb: wqkv_sb[cb][:, 2 * C + off:2 * C + off + width],
                    lambda pt: nc.vector.tensor_copy(
                        vo_sb[nt].rearrange("p (h e) -> p h e", e=Dh + 1)[:, h0:h0 + hn, 0:Dh],
                        pt[:, 0:width].rearrange("p (h e) -> p h e", e=Dh)),
                    f"v{nt}{chunk}",
                    width=width,
                )

            def emit_all(parts):
                for fn in parts:
                    fn()

            # ---- prefix: band-4 qT + kt0-covering kT slice; scores can
            # start as soon as the query columns of x land.
            emit_all(qT_parts(4, 0))
            emit_all(qT_parts(4, 1))
            emit_all(kT_parts(4, 0, 256))

            # ---- filler queue, deadline-ordered for pair order 4,5,0,1,2,3.
            # band_end[b] = filler index that must be emitted before band-b
            # scores (reads must not precede writes in emission order).
            v_emit = [[-1] * NT, [-1] * NT]  # [chunk][nt] -> filler index
            fillers = []  # (emit_fn, (chunk, nt) completed or None)
            band_end = [0] * CB

            def add(parts, done=None):
                for fn in parts[:-1]:
                    fillers.append((fn, None))
                fillers.append((parts[-1], done))

            add(kT_parts(4, 256, 256))
            add(kT_parts(4, 512))
            for nt in range(4):
                add(v_parts(nt, 1), (1, nt))
            add(kT_parts(4, 1024))
            add(kT_parts(4, 1536))
            for nt in range(4, NT):
                add(v_parts(nt, 1), (1, nt))
            for b in (5, 0):
                for kc in range(2):
                    for half in range(2):
                        add(kT_parts(b, kc * 1024 + half * 512))
                add(qT_parts(b, 0))
                add(qT_parts(b, 1))
                band_end[b] = len(fillers)
            for nt in range(NT):
                add(v_parts(nt, 0), (0, nt))
            for b in (1, 2, 3):
                for kc in range(2):
                    for half in range(2):
                        add(kT_parts(b, kc * 1024 + half * 512))
                add(qT_parts(b, 0))
                add(qT_parts(b, 1))
                band_end[b] = len(fillers)
            fill_i = [0]

            def pop_fill(k=1):
                for _ in range(k):
                    if fill_i[0] < len(fillers):
                        fn, done = fillers[fill_i[0]]
                        fn()
                        if done is not None:
                            v_emit[done[0]][done[1]] = fill_i[0]
                        fill_i[0] += 1

            def pop_fill_until(idx):
                while fill_i[0] < idx:
                    pop_fill(1)

            # ---- attention state ----
            uts = {}   # (h, kt) -> ut tile
            pus = {}   # h -> pu accumulator

            def emit_scores(p, kt):
                band = p
                psA = ps_a.tile([P, 1024], F32, name="psA", tag="ps")
                for qc in range(QC):
                    nc.tensor.matmul(
                        psA[:, qc * 512:(qc + 1) * 512],
                        kT_sb[band][0:64, kt * P:(kt + 1) * P],
                        qT_sb[band][0:64, qc * 512:(qc + 1) * 512],
                        start=True,
                        stop=True,
                    )
                psB = ps_b.tile([P, 1024], F32, name="psB", tag="ps")
                for qc in range(QC):
                    nc.tensor.matmul(
                        psB[:, qc * 512:(qc + 1) * 512],
                        kT_sb[band][64:128, kt * P:(kt + 1) * P],
                        qT_sb[band][64:128, qc * 512:(qc + 1) * 512],
                        start=True,
                        stop=True,
                    )
                utA = pool_ua.tile([P, 1024], BF16, name="utA", tag="ua")
                nc.scalar.activation(utA[:], psA[:], Exp)
                utB = pool_ub.tile([P, 1024], BF16, name="utB", tag="ub")
                nc.scalar.activation(utB[:], psB[:], Exp)
                uts[(2 * p, kt)] = utA
                uts[(2 * p + 1, kt)] = utB
                if DBG and p == 4 and kt == 0:
                    nc.sync.dma_start(dbg_ut.ap()[0], utA[:])
                    nc.sync.dma_start(dbg_ut.ap()[1], utB[:])

            def emit_av(h, kt):
                if kt == 0:
                    pus[h] = ps_u.tile([65, 1024], F32, name="pu")
                pu = pus[h]
                for qc in range(QC):
                    nc.tensor.matmul(
                        pu[0:65, qc * 512:(qc + 1) * 512],
                        vo_sb[kt][:, h * 65:(h + 1) * 65],
                        uts[(h, kt)][:, qc * 512:(qc + 1) * 512],
                        start=(kt == 0),
                        stop=(kt == NT - 1),
                    )
                del uts[(h, kt)]

            def emit_normalize(h):
                # copy pu -> SBUF staging first: frees the single pu PSUM
                # buffer after ~1.2us instead of after the full chain
                band, hp = divmod(h, 2)
                po = hp * 64
                pu = pus.pop(h)
                st = pool_st.tile([64, ROWS], BF16, name="st")
                nc.vector.tensor_copy(st[:], pu[0:64, :])
                s = pool_r.tile([1, ROWS], F32, name="s", tag="r")
                nc.vector.tensor_copy(s[:], pu[64:65, :])
                r = pool_r.tile([1, ROWS], F32, name="r", tag="r")
                nc.vector.reciprocal_approx_fast(r[:], s[:])
                rb = pool_rb.tile([64, ROWS], F32, name="rb")
                nc.gpsimd.partition_broadcast(rb[:], r[:])
                nc.vector.tensor_mul(attT_sb[band][po:po + 64, :], st[:], rb[:])

            # ---- main loop. Per step: AV rides FIRST (deadlock-safe: a
            # stalled ride can never block the exp that frees its ut slot),
            # then filler, then scores (phase-offset exps keep ACT gap-free).
            # pu (single PSUM accumulator) rotation per pair p:
            #   step 0:    finish head A(p-1) spill kts, copy pu -> staging
            #   steps 1-8: ride head B(p-1), 2 kts/step; then copy/normalize
            #   steps 9-15: self-ride head A(p) (its exps are >=1 step old)
            av_done = [0] * H     # kts consumed per head
            norm_done = [False] * H

            def finish(h):
                if not norm_done[h]:
                    emit_normalize(h)
                    norm_done[h] = True

            def ride(h, want, kt_cap=NT):
                # emit up to `want` AV kts for head h; the vo chunk this head
                # lives in must have been emitted; self-rides capped at kt<step
                chunk = 0 if h < 8 else 1
                drained = fill_i[0] >= len(fillers)
                n = 0
                while n < want and av_done[h] < min(NT, kt_cap):
                    kt = av_done[h]
                    if v_emit[chunk][kt] < 0:
                        break
                    if not drained and fill_i[0] - v_emit[chunk][kt] < 2:
                        break
                    emit_av(h, kt)
                    av_done[h] += 1
                    n += 1
                if av_done[h] == NT:
                    finish(h)

            def force(h):
                # unconditional completion (the band deadlines guarantee the
                # vo chains for h were already emitted)
                while av_done[h] < NT:
                    emit_av(h, av_done[h])
                    av_done[h] += 1
                finish(h)

            PAIR_ORDER = [4, 5, 0, 1, 2, 3]
            cumB = [0, 3, 6, 9, 12, 14, 16]  # hB(prev) ride targets, steps 1-6
            for pi, p in enumerate(PAIR_ORDER):
                hA, hB = 2 * p, 2 * p + 1
                prev = PAIR_ORDER[pi - 1] if pi >= 1 else None
                prev2 = PAIR_ORDER[pi - 2] if pi >= 2 else None
                hAp = 2 * prev if prev is not None else None
                hBp = 2 * prev + 1 if prev is not None else None
                pop_fill_until(band_end[p])  # band deadline (usually no-op)
                for kt in range(NT):
                    # scores first: keeps the phase-offset exps gap-free
                    emit_scores(p, kt)
                    rode = 0
                    if pi > 0:
                        if kt == 0:
                            if prev2 is not None:
                                force(2 * prev2 + 1)  # safety: pu order
                            force(hAp)  # head A(prev) spill (kt 15)
                            rode = 2
                        elif kt <= 6:
                            n0 = av_done[hBp]
                            ride(hBp, max(0, cumB[kt] - n0))
                            rode += av_done[hBp] - n0
                        elif av_done[hBp] < NT:
                            n0 = av_done[hBp]
                            ride(hBp, NT)  # catch-up if vo gating lagged
                            rode += av_done[hBp] - n0
                    if kt >= 7 and (norm_done[hBp] if pi > 0 else True):
                        n0 = av_done[hA]
                        want = 2 * (kt - 6) - n0
                        ride(hA, max(0, want), kt_cap=kt + 1)
                        rode += av_done[hA] - n0
                    # adaptive filler pops: ~2.2us/step PE budget
                    if pi == 0 and kt <= 6:
                        pop_fill(2)
                    else:
                        pop_fill(1 if rode >= 2 else 2)

            # ---- epilogue: finish last pair, overlap proj partials ----
            pop_fill(len(fillers))  # drain any leftovers
            force(2 * PAIR_ORDER[-2] + 1)
            pl = PAIR_ORDER[-1]
            hA, hB = 2 * pl, 2 * pl + 1
            while av_done[hA] < NT:
                emit_av(hA, av_done[hA])
                av_done[hA] += 1
            finish(hA)

            if DBG:
                for cb in range(CB):
                    nc.sync.dma_start(dbg_qT.ap()[cb * P:(cb + 1) * P, :], qT_sb[cb][:])
                    nc.sync.dma_start(dbg_kT.ap()[cb * P:(cb + 1) * P, :], kT_sb[cb][:])
                    nc.sync.dma_start(dbg_att.ap()[cb * P:(cb + 1) * P, :], attT_sb[cb][:])
                for nt in range(NT):
                    nc.sync.dma_start(dbg_vo.ap()[nt * P:(nt + 1) * P, :], vo_sb[nt][:])

            # ---- output projection: 12 chains in 2 groups of 6 (ps_f x2 +
            # ps_a + ps_b give 6 concurrent accumulators once scores stop).
            # Contraction order puts band 3 (normalized last) at the end of
            # each chain; group 0's partials interleave with h7's AV.
            PROJ_CB = (4, 5, 0, 1, 2, 3)
            chains = [(ob, qc) for ob in range(CB) for qc in range(QC)]

            def proj_mm(slot, ob, qc, i):
                cb = PROJ_CB[i]
                nc.tensor.matmul(
                    slot,
                    wp_sb[cb][:, ob * P:(ob + 1) * P],
                    attT_sb[cb][:, qc * 512:(qc + 1) * 512],
                    start=(i == 0),
                    stop=(i == CB - 1),
                )

            def proj_group(g, ride_h=None):
                base = 6 * g
                tf0 = ps_f.tile([P, 512], F32, name=f"pj{base}", tag="fill")
                tf1 = ps_f.tile([P, 512], F32, name=f"pj{base + 1}", tag="fill")
                ta = ps_a.tile([P, 1024], F32, name="pja", tag="ps")
                tb = ps_b.tile([P, 1024], F32, name="pjb", tag="ps")
                slots = [tf0[:], tf1[:], ta[:, 0:512], ta[:, 512:1024],
                         tb[:, 0:512], tb[:, 512:1024]]
                for j in range(6):
                    ob, qc = chains[base + j]
                    for i in range(5):
                        proj_mm(slots[j], ob, qc, i)
                    if ride_h is not None:
                        for _ in range(3):
                            if av_done[ride_h] < NT:
                                emit_av(ride_h, av_done[ride_h])
                                av_done[ride_h] += 1
                if ride_h is not None:
                    finish(ride_h)
                for j in range(6):
                    ob, qc = chains[base + j]
                    proj_mm(slots[j], ob, qc, 5)
                    y = pool_y.tile([P, 512], F32, name="y")
                    nc.vector.tensor_scalar_add(y[:], slots[j], bias_sb[ob][:])
                    nc.sync.dma_start(
                        out_d.ap()[ob * P:(ob + 1) * P, qc * 512:(qc + 1) * 512], y[:]
                    )

            proj_group(0, ride_h=hB)
            proj_group(1)

    nc.compile()
    return nc


def kernel(x, w_qkv, w_proj, b_proj):
    global _cached_nc, LAST_RESULT
    if _cached_nc is None:
        _cached_nc = _build_nc()
    nc = _cached_nc

    x = np.asarray(x, dtype=np.float32)
    w_qkv = np.asarray(w_qkv, dtype=np.float32)
    w_proj = np.asarray(w_proj, dtype=np.float32)
    b_proj = np.asarray(b_proj, dtype=np.float32)

    bf = ml_dtypes.bfloat16
    wqkvT = w_qkv.T.astype(np.float32).copy()  # [C, 3C]
    wqkvT[:, :C] *= SCALE  # fold q scaling
    wqkvT = np.ascontiguousarray(wqkvT).astype(bf)
    wprojT = np.ascontiguousarray(w_proj.T).astype(bf)
    bproj_dev = np.ascontiguousarray(b_proj.astype(np.float32).reshape(CB, P, 1))

    in_maps = []
    for c in range(NCORES):
        b, half = divmod(c, 2)
        xTb = x[b].T.astype(bf)  # [C, N]
        if half:
            xTb = np.roll(xTb, -ROWS, axis=1)  # query rows -> columns 0:1024
        in_maps.append(
            {
                "xT": np.ascontiguousarray(xTb),
                "wqkvT": wqkvT,
                "wprojT": wprojT,
                "bproj": bproj_dev,
            }
        )

    res = bass_utils.run_bass_kernel_spmd(nc, in_maps, core_ids=list(range(NCORES)))
    LAST_RESULT = res

    out = np.empty((B, N, C), np.float32)
    for c in range(NCORES):
        b, half = divmod(c, 2)
        out[b, half * ROWS:(half + 1) * ROWS, :] = res.results[c]["out"].T
    return out


# revision 42
# speedup vs baseline: 1.2156x; 1.0264x over previous
"""Multi-head attention (B=4, N=2048, C=768, H=12, D=64) on 8 TRN2 NeuronCores.

Sharding: core c handles batch c//2 and query rows (c%2)*1024 .. +1024, all
heads. Each core recomputes K/V for its full batch (cheaper than any
collective), so there is no cross-core communication at all. The host ROTATES
x[b].T per core so that this core's query rows are always columns 0:1024 —
attention is permutation-invariant over keys, so k/v built from the rotated
sequence give identical results and the SPMD graph stays core-independent.

Layouts (host pre-transposes; contraction dim always on SBUF partitions):
  qT/kT = (wqkvT.T @ xT-slices), v natural = xT-tile.T @ wvT, packed per
  key-tile as [v_h | ones] so the AV matmul also produces the softmax
  denominator in PSUM row 64. scoresT[keys, q] = kT_tile.T @ qT; exp on
  ScalarE over [128, 1024] PSUM tiles (two N=512 matmuls fill one tile; the
  wide activation amortizes ACT's ~352-cycle per-instruction overhead); no max
  subtraction (scores are O(1) by construction). AV matmuls for head h-1
  interleave with scores/exp of head h: their inputs are all ready, so the PE
  stream has no ACT-dependent stalls. Normalization: reciprocal_approx_fast on
  the sums row, gpsimd partition-broadcast, fused into the PSUM->SBUF evict.
The 1/sqrt(D) scale is folded into the q columns of wqkvT on the host.
"""

from contextlib import ExitStack

import ml_dtypes
import numpy as np

import concourse.bass as bass
import concourse.tile as tile
from concourse import bacc, mybir
from concourse import bass_utils

B, N, C, H, Dh = 4, 2048, 768, 12, 64
P = 128
NCORES = 8
ROWS = N // 2  # query rows per core
SCALE = Dh ** -0.5

BF16 = mybir.dt.bfloat16
F32 = mybir.dt.float32

CB = C // P       # 6 contraction bands
NT = N // P       # 16 key tiles
QC = ROWS // 512  # 2 query half-chunks (N=512 matmuls)
KCH = N // 1024   # 2 key eviction chunks for kT

_cached_nc = None
LAST_RESULT = None  # BassKernelResults of the most recent run (for test harness)


def _build_nc():
    nc = bacc.Bacc(
        "TRN2",
        target_bir_lowering=False,
        debug=False,
        enable_asserts=False,
        num_devices=NCORES,
    )
    xT_d = nc.dram_tensor("xT", [C, N], BF16, kind="ExternalInput")
    wqkvT_d = nc.dram_tensor("wqkvT", [C, 3 * C], BF16, kind="ExternalInput")
    wprojT_d = nc.dram_tensor("wprojT", [C, C], BF16, kind="ExternalInput")
    bproj_d = nc.dram_tensor("bproj", [CB, P, 1], F32, kind="ExternalInput")
    out_d = nc.dram_tensor("out", [C, ROWS], F32, kind="ExternalOutput")

    Exp = mybir.ActivationFunctionType.Exp

    with tile.TileContext(nc) as tc:
        with ExitStack() as ctx:
            # ---- persistent pools ----
            pool_wp = ctx.enter_context(tc.tile_pool(name="wproj", bufs=1))
            pool_bias = ctx.enter_context(tc.tile_pool(name="bias", bufs=1))
            pool_qT = ctx.enter_context(tc.tile_pool(name="qT", bufs=1))
            pool_kT = ctx.enter_context(tc.tile_pool(name="kT", bufs=1))
            pool_vo = ctx.enter_context(tc.tile_pool(name="vones", bufs=1))
            pool_attT = ctx.enter_context(tc.tile_pool(name="attT", bufs=1))

            wp_sb = [pool_wp.tile([P, C], BF16, name=f"wp{i}") for i in range(CB)]
            bias_sb = [pool_bias.tile([P, 1], F32, name=f"bias{i}") for i in range(CB)]
            qT_sb = [pool_qT.tile([P, ROWS], BF16, name=f"qT{i}") for i in range(CB)]
            kT_sb = [pool_kT.tile([P, N], BF16, name=f"kT{i}") for i in range(CB)]
            # per key-tile: 12 heads x [v_h (64 cols) | ones (1 col)]
            vo_sb = [pool_vo.tile([P, H * (Dh + 1)], BF16, name=f"vo{i}") for i in range(NT)]
            attT_sb = [pool_attT.tile([P, ROWS], BF16, name=f"attT{i}") for i in range(CB)]

            for nt in range(NT):
                nc.gpsimd.memset(vo_sb[nt][:], 1.0)

            # ---- stage 1: qkv projections (own scope; pools freed after) ----
            with ExitStack() as s1:
                pool_x = s1.enter_context(tc.tile_pool(name="xT", bufs=1))
                pool_wqkv = s1.enter_context(tc.tile_pool(name="wqkv", bufs=1))
                ps_qk = s1.enter_context(tc.tile_pool(name="ps_qk", bufs=2, space="PSUM"))
                ps_v = s1.enter_context(tc.tile_pool(name="ps_v", bufs=2, space="PSUM"))

                x_sb = [pool_x.tile([P, N], BF16, name=f"x{i}") for i in range(CB)]
                wqkv_sb = [pool_wqkv.tile([P, 3 * C], BF16, name=f"wqkv{i}") for i in range(CB)]
                # priority order: q-columns of x + q-section of wqkv first so the
                # first projection chains start ~7us earlier; bulk follows.
                for cb in range(CB):
                    nc.sync.dma_start(x_sb[cb][:, 0:ROWS], xT_d.ap()[cb * P:(cb + 1) * P, 0:ROWS])
                    nc.sync.dma_start(wqkv_sb[cb][:, 0:2 * P], wqkvT_d.ap()[cb * P:(cb + 1) * P, 0:2 * P])
                for cb in range(CB):
                    nc.sync.dma_start(wqkv_sb[cb][:, 2 * P:C], wqkvT_d.ap()[cb * P:(cb + 1) * P, 2 * P:C])
                for cb in range(CB):
                    nc.sync.dma_start(x_sb[cb][:, ROWS:N], xT_d.ap()[cb * P:(cb + 1) * P, ROWS:N])
                    nc.sync.dma_start(wqkv_sb[cb][:, C:3 * C], wqkvT_d.ap()[cb * P:(cb + 1) * P, C:3 * C])
                for cb in range(CB):
                    nc.sync.dma_start(wp_sb[cb][:], wprojT_d.ap()[cb * P:(cb + 1) * P, :])
                    nc.sync.dma_start(bias_sb[cb][:], bproj_d.ap()[cb, :, :])

                # qT[o, n]: this core's query rows = x columns 0:1024 (host-rotated)
                for ob in range(CB):
                    pt = ps_qk.tile([P, 1024], F32, name="pt_q", tag="pt_qk")
                    for cb in range(CB):  # cb outer: both qc halves share one stationary
                        for qc in range(QC):
                            nc.tensor.matmul(
                                pt[:, qc * 512:(qc + 1) * 512],
                                wqkv_sb[cb][:, ob * P:(ob + 1) * P],
                                x_sb[cb][:, qc * 512:(qc + 1) * 512],
                                start=(cb == 0),
                                stop=(cb == CB - 1),
                            )
                    nc.vector.tensor_copy(qT_sb[ob][:], pt[:])
                # kT[o, n]: kc=0 chains first (their x columns arrive earlier)
                for kc in range(KCH):
                    for ob in range(CB):
                        pt = ps_qk.tile([P, 1024], F32, name="pt_k", tag="pt_qk")
                        for cb in range(CB):  # cb outer: halves share one stationary
                            for half in range(2):
                                nc.tensor.matmul(
                                    pt[:, half * 512:(half + 1) * 512],
                                    wqkv_sb[cb][:, C + ob * P:C + (ob + 1) * P],
                                    x_sb[cb][:, kc * 1024 + half * 512:kc * 1024 + (half + 1) * 512],
                                    start=(cb == 0),
                                    stop=(cb == CB - 1),
                                )
                        nc.vector.tensor_copy(kT_sb[ob][:, kc * 1024:(kc + 1) * 1024], pt[:])
                # v natural [n, c]; evict all 12 heads at once via 3D AP into [v|1] tiles
                for nt in range(NT):
                    pt = ps_v.tile([P, C], F32, name="pt_v")
                    for cb in range(CB):  # cb outer: x-tile stationary shared by chunks
                        for off, width in ((0, 512), (512, 256)):  # bank-aligned
                            nc.tensor.matmul(
                                pt[:, off:off + width],
                                x_sb[cb][:, nt * P:(nt + 1) * P],
                                wqkv_sb[cb][:, 2 * C + off:2 * C + off + width],
                                start=(cb == 0),
                                stop=(cb == CB - 1),
                            )
                    nc.vector.tensor_copy(
                        vo_sb[nt].rearrange("p (h e) -> p h e", e=Dh + 1)[:, :, 0:Dh],
                        pt[:].rearrange("p (h e) -> p h e", e=Dh),
                    )

            # ---- stage 2: attention; AV of head h-1 rides behind scores/exp of h ----
            pool_u = ctx.enter_context(tc.tile_pool(name="u", bufs=40))
            pool_r = ctx.enter_context(tc.tile_pool(name="r", bufs=4))
            pool_rb = ctx.enter_context(tc.tile_pool(name="rb", bufs=4))
            pool_y = ctx.enter_context(tc.tile_pool(name="y", bufs=3))
            ps_s = ctx.enter_context(tc.tile_pool(name="ps_s", bufs=2, space="PSUM"))
            ps_u = ctx.enter_context(tc.tile_pool(name="ps_u", bufs=2, space="PSUM"))

            uts = {}   # (h, kt) -> uT tile
            pus = {}   # h -> pu accumulator tile

            def emit_scores(h):
                band, hp = divmod(h, 2)
                po = hp * 64
                for kt in range(NT):
                    ps = ps_s.tile([P, 1024], F32, name="ps")
                    for qc in range(QC):
                        nc.tensor.matmul(
                            ps[:, qc * 512:(qc + 1) * 512],
                            kT_sb[band][po:po + 64, kt * P:(kt + 1) * P],
                            qT_sb[band][po:po + 64, qc * 512:(qc + 1) * 512],
                            start=True,
                            stop=True,
                        )
                    ut = pool_u.tile([P, 1024], BF16, name="ut")
                    nc.scalar.activation(ut[:], ps[:], Exp)
                    uts[(h, kt)] = ut
                    yield

            def emit_av(h):
                pu = ps_u.tile([P, 1024], F32, name="pu")
                pus[h] = pu
                for kt in range(NT):
                    for qc in range(QC):
                        nc.tensor.matmul(
                            pu[0:65, qc * 512:(qc + 1) * 512],
                            vo_sb[kt][:, h * 65:(h + 1) * 65],
                            uts[(h, kt)][:, qc * 512:(qc + 1) * 512],
                            start=(kt == 0),
                            stop=(kt == NT - 1),
                        )
                    yield

            def emit_normalize(h, split=False):
                band, hp = divmod(h, 2)
                po = hp * 64
                pu = pus.pop(h)
                s = pool_r.tile([1, ROWS], F32, name="s", tag="r")
                nc.vector.tensor_copy(s[:], pu[64:65, :])
                r = pool_r.tile([1, ROWS], F32, name="r", tag="r")
                nc.vector.reciprocal_approx_fast(r[:], s[:])
                rb = pool_rb.tile([64, ROWS], F32, name="rb")
                if split:  # halves pipelined so proj's first chunk unblocks sooner
                    for qc in range(QC):
                        sl = slice(qc * 512, (qc + 1) * 512)
                        nc.gpsimd.partition_broadcast(rb[:, sl], r[:, sl])
                        nc.vector.tensor_mul(
                            attT_sb[band][po:po + 64, sl], pu[0:64, sl], rb[:, sl]
                        )
                else:
                    nc.gpsimd.partition_broadcast(rb[:], r[:])
                    nc.vector.tensor_mul(attT_sb[band][po:po + 64, :], pu[0:64, :], rb[:])
                for kt in range(NT):
                    del uts[(h, kt)]

            for h in range(H):
                sc = emit_scores(h)
                av = emit_av(h - 1) if h > 0 else None
                for kt in range(0, NT, 2):  # kt-pair granularity halves PSUM
                    next(sc)                # bank-group switches on the PE
                    next(sc)
                    if av is not None:
                        next(av, None)
                        next(av, None)
                if av is not None:
                    emit_normalize(h - 1)
            for _ in emit_av(H - 1):
                pass
            emit_normalize(H - 1, split=True)

            # ---- stage 3: output projection (psum shared with ps_u slots) ----
            for ob in range(CB):
                for qc in range(QC):
                    pt = ps_s.tile([P, 512], F32, name="pt_y", tag="ps")
                    for cb in range(CB):
                        nc.tensor.matmul(
                            pt[:],
                            wp_sb[cb][:, ob * P:(ob + 1) * P],
                            attT_sb[cb][:, qc * 512:(qc + 1) * 512],
                            start=(cb == 0),
                            stop=(cb == CB - 1),
                        )
                    y = pool_y.tile([P, 512], F32, name="y")
                    nc.vector.tensor_scalar_add(y[:], pt[:], bias_sb[ob][:])
                    nc.sync.dma_start(
                        out_d.ap()[ob * P:(ob + 1) * P, qc * 512:(qc + 1) * 512], y[:]
                    )

    nc.compile()
    return nc


def kernel(x, w_qkv, w_proj, b_proj):
    global _cached_nc, LAST_RESULT
    if _cached_nc is None:
        _cached_nc = _build_nc()
    nc = _cached_nc

    x = np.asarray(x, dtype=np.float32)
    w_qkv = np.asarray(w_qkv, dtype=np.float32)
    w_proj = np.asarray(w_proj, dtype=np.float32)
    b_proj = np.asarray(b_proj, dtype=np.float32)

    bf = ml_dtypes.bfloat16
    wqkvT = w_qkv.T.astype(np.float32).copy()  # [C, 3C]
    wqkvT[:, :C] *= SCALE  # fold q scaling
    wqkvT = np.ascontiguousarray(wqkvT).astype(bf)
    wprojT = np.ascontiguousarray(w_proj.T).astype(bf)
    bproj_dev = np.ascontiguousarray(b_proj.astype(np.float32).reshape(CB, P, 1))

    in_maps = []
    for c in range(NCORES):
        b, half = divmod(c, 2)
        xTb = x[b].T.astype(bf)  # [C, N]
        if half:
            xTb = np.roll(xTb, -ROWS, axis=1)  # query rows -> columns 0:1024
        in_maps.append(
            {
                "xT": np.ascontiguousarray(xTb),
                "wqkvT": wqkvT,
                "wprojT": wprojT,
                "bproj": bproj_dev,
            }
        )

    res = bass_utils.run_bass_kernel_spmd(nc, in_maps, core_ids=list(range(NCORES)))
    LAST_RESULT = res

    out = np.empty((B, N, C), np.float32)
    for c in range(NCORES):
        b, half = divmod(c, 2)
        out[b, half * ROWS:(half + 1) * ROWS, :] = res.results[c]["out"].T
    return out

